# revision 1
# baseline (speedup 1.0000x reference)
"""Trainium2 Bass kernel for quantized dense layer with Hadamard rotations.

Math (see reference): y = (H2 @ (sq(H2@x) @ sq(w@H1)) @ H1)/(64*64) + bias,
where sq() is per-tensor symmetric int8 stochastic quantization.

Structure exploited: Sylvester Hadamards factor as Kronecker products
(H4096 = H32 (x) H128).  Every Hadamard application is a per-128-tile PE
matmul against an H128 constant plus a cross-tile DVE add/sub butterfly.
The core GEMM runs on int8-valued operands stored as bf16 (exact:
|acc| < 2^25) at full PE rate.  Stochastic rounding is computed as
rint(x*scale + (0.5 - noise)) via the fp32->int32 round-to-nearest cast,
with (0.5 - noise) precomputed on the host.

Sharding (8 cores): the IN axis is split 8 ways for forward transforms +
quantization (column/row-local).  Global quant scales via two 1-scalar
AllReduce-max ops (split so x-quant + AllGather overlap the w transform).
Quantized operands are PE-transposed into collective-friendly layouts,
exchanged via AllGather (activations) and AllToAll (weights).  Each core
computes yr[:, Fk] and applies every inverse-transform factor except the
outer H8 on features, which is folded into the host-side unshard (an 8x8
combine over gathered per-core outputs).
"""
import sys, os
sys.path.insert(0, '/opt/trn_rl_repo')
import numpy as np

B, IN, F = 4096, 2048, 4096
NCORES = 8
CS = IN // NCORES      # 256  per-core IN slice
FS = F // NCORES       # 512  per-core feature block
BT = B // 128          # 32   batch tiles
KT = IN // 128         # 16   contraction tiles
FT = FS // 128         # 4    feature tiles in a block
QMAX = 127.0
KSTOP = int(os.environ.get("KSTOP", "8"))

_cache = {}


class _StopBuild(Exception):
    pass


def _sylvester(n):
    h = np.array([[1.0]], dtype=np.float32)
    while h.shape[0] < n:
        h = np.block([[h, h], [h, -h]])
    return h


def _build():
    from concourse import bass, bacc, tile, mybir
    import concourse.bass_isa as bass_isa

    DT = mybir.dt.float32
    BF = mybir.dt.bfloat16
    I32 = mybir.dt.int32
    A = mybir.AluOpType
    npbf = mybir.dt.np(BF)

    nc = bacc.Bacc("TRN2", target_bir_lowering=False, debug=False,
                   num_devices=NCORES)

    xk = nc.dram_tensor("xk", [B, CS], DT, kind="ExternalInput")
    nk = nc.dram_tensor("nk", [B, CS], DT, kind="ExternalInput")   # 0.5-noise_x
    wk = nc.dram_tensor("wk", [F, CS], DT, kind="ExternalInput")   # w slice^T
    mk = nc.dram_tensor("mk", [F, CS], DT, kind="ExternalInput")   # (.5-noise_w)^T
    out = nc.dram_tensor("out", [FS, B], DT, kind="ExternalOutput")

    sx_i = nc.dram_tensor("sx_i", [1, 1], DT)
    sx_o = nc.dram_tensor("sx_o", [1, 1], DT, addr_space="Shared")
    sw_i = nc.dram_tensor("sw_i", [1, 1], DT)
    sw_o = nc.dram_tensor("sw_o", [1, 1], DT, addr_space="Shared")
    xqc = nc.dram_tensor("xqc", [CS, B], BF)                       # xq^T slice
    xqg = nc.dram_tensor("xqg", [IN, B], BF, addr_space="Shared")  # full xq^T
    wac = nc.dram_tensor("wac", [IN, FS], BF)                      # A2A contrib
    wblk = nc.dram_tensor("wblk", [IN, FS], BF)                    # wq[:, Fk]

    h128f_d = nc.inline_tensor(_sylvester(128), name="h128f")
    h128b_d = nc.inline_tensor(_sylvester(128).astype(npbf), name="h128b")
    idb_d = nc.inline_tensor(np.eye(128, dtype=np.float32).astype(npbf),
                             name="idb")
    rg = [list(range(NCORES))]

    NB = 32 * CS  # 8192 free columns in a fwd big tile

    def butterfly(nc, bufs, T, blk0, A):
        """FWHT across the tile-index axis of big tensors [128, T*blk0].
        Emitted as 2D contiguous ops (per hi-group) for DVE fast modes."""
        n = T.bit_length() - 1
        for s in range(n):
            cur, nxt = bufs(s)
            blk = blk0 << s
            hi = T >> (s + 1)
            for h in range(hi):
                a0 = h * 2 * blk
                a1 = a0 + blk
                nc.vector.tensor_tensor(nxt[:, a0:a0 + blk],
                                        cur[:, a0:a0 + blk],
                                        cur[:, a1:a1 + blk], op=A.add)
                nc.vector.tensor_tensor(nxt[:, a1:a1 + blk],
                                        cur[:, a0:a0 + blk],
                                        cur[:, a1:a1 + blk], op=A.subtract)

    with tile.TileContext(nc) as tc:
      try:
        with tc.tile_pool(name="consts", bufs=1) as cpool:
            h128f = cpool.tile([128, 128], DT)
            h128b = cpool.tile([128, 128], BF)
            idb = cpool.tile([128, 128], BF)
            nc.sync.dma_start(h128f[:], h128f_d[:])
            nc.sync.dma_start(h128b[:], h128b_d[:])
            nc.sync.dma_start(idb[:], idb_d[:])

            # ================= forward transforms + quant =================
            with tc.tile_pool(name="fwd", bufs=2) as fp_, \
                 tc.tile_pool(name="fin", bufs=4) as fin, \
                 tc.tile_pool(name="fps", bufs=1, space="PSUM") as fps, \
                 tc.tile_pool(name="qtmp", bufs=2) as qtmp, \
                 tc.tile_pool(name="qT", bufs=4) as qTp, \
                 tc.tile_pool(name="qsc", bufs=1) as qsc:

                def fwd_side(src_tile_ap, ntiles, side):
                    bigA = fp_.tile([128, NB], DT, tag="bigA",
                                    name=f"bigA{side}")
                    bigB = fp_.tile([128, NB], DT, tag="bigB",
                                    name=f"bigB{side}")
                    for o in range(ntiles):
                        t = fin.tile([128, CS], DT, tag="fin", name="fint")
                        nc.sync.dma_start(t[:], src_tile_ap(o))
                        ps = fps.tile([128, CS], DT, tag="ps", name="fpst",
                                      bufs=4)
                        nc.tensor.matmul(ps[:], h128f[:], t[:], start=True,
                                         stop=True)
                        nc.vector.tensor_copy(bigA[:, o * CS:(o + 1) * CS],
                                              ps[:])
                    butterfly(nc, lambda s: (bigA, bigB) if s % 2 == 0
                              else (bigB, bigA), 32, CS, A)
                    return bigB  # 5 stages -> result in B

                def scale_trigger(big, tag, cc_in, cc_out):
                    am = qsc.tile([128, 1], DT, tag=f"am{tag}",
                                  name=f"am{tag}")
                    nc.vector.tensor_reduce(am[:], big[:],
                                            axis=mybir.AxisListType.X,
                                            op=A.max,
                                            apply_absolute_value=True)
                    red = qsc.tile([128, 1], DT, tag=f"rd{tag}",
                                   name=f"rd{tag}")
                    nc.gpsimd.partition_all_reduce(
                        red[:], am[:], channels=128,
                        reduce_op=bass_isa.ReduceOp.absmax)
                    nc.sync.dma_start(cc_in[:], red[0:1, 0:1])
                    nc.gpsimd.collective_compute(
                        "AllReduce", A.max, replica_groups=rg,
                        ins=[cc_in.ap().opt()], outs=[cc_out.ap().opt()])

                def scale_finish(tag, cc_out):
                    sg = qsc.tile([1, 1], DT, tag=f"sg{tag}",
                                  name=f"sg{tag}")
                    nc.sync.dma_start(sg[0:1, :], cc_out[:])
                    # r = QMAX/s with one newton step
                    r0 = qsc.tile([1, 1], DT, tag=f"r0{tag}", name=f"r0{tag}")
                    nc.vector.reciprocal(r0[0:1, :], sg[0:1, :])
                    mr = qsc.tile([1, 1], DT, tag=f"mr{tag}", name=f"mr{tag}")
                    nc.vector.tensor_tensor(mr[0:1, :], sg[0:1, :],
                                            r0[0:1, :], op=A.mult)
                    tw = qsc.tile([1, 1], DT, tag=f"tw{tag}", name=f"tw{tag}")
                    nc.vector.tensor_scalar(tw[0:1, :], mr[0:1, :], -1.0, 2.0,
                                            op0=A.mult, op1=A.add)
                    r1 = qsc.tile([1, 1], DT, tag=f"r1{tag}", name=f"r1{tag}")
                    nc.vector.tensor_tensor(r1[0:1, :], r0[0:1, :],
                                            tw[0:1, :], op=A.mult)
                    r127 = qsc.tile([1, 1], DT, tag=f"rq{tag}",
                                    name=f"rq{tag}")
                    nc.vector.tensor_scalar_mul(r127[0:1, :], r1[0:1, :],
                                                QMAX)
                    rb = qsc.tile([128, 1], DT, tag=f"rb{tag}",
                                  name=f"rb{tag}")
                    nc.gpsimd.partition_broadcast(rb[:, 0:1], r127[0:1, 0:1])
                    return sg, rb

                CH = 1024   # quant chunk = 4 o-tiles

                def quant_transpose(big, rb, noise_ap, side, tiles_T):
                    """quantize [128, NB] -> int-valued bf16, PE-transpose
                    128-blocks into tiles_T[h][128, B] (h = col-half)."""
                    nt_ch = CH // CS  # 4
                    for ch in range(NB // CH):
                        nz = qtmp.tile([128, CH], DT, tag="nz", name="nzt")
                        nc.sync.dma_start(
                            nz[:].rearrange("p (o c) -> p o c", o=nt_ch),
                            noise_ap(ch))
                        qi = qtmp.tile([128, CH], I32, tag="qi", name="qit")
                        nc.vector.scalar_tensor_tensor(
                            qi[:], big[:, ch * CH:(ch + 1) * CH], rb[:, 0:1],
                            nz[:], op0=A.mult, op1=A.add)
                        qb = qtmp.tile([128, CH], BF, tag=f"qb{side}",
                                       name="qbt")
                        nc.vector.tensor_copy(qb[:], qi[:])
                        for ol in range(nt_ch):
                            o = ch * nt_ch + ol
                            for h in range(2):
                                ps = fps.tile([128, 128], BF, tag="tps",
                                              name="tpst", bufs=4)
                                nc.tensor.transpose(
                                    ps[:],
                                    qb[:, ol * CS + h * 128:
                                       ol * CS + (h + 1) * 128], idb[:])
                                eng = nc.scalar if (o + h) % 2 else nc.vector
                                if eng is nc.scalar:
                                    nc.scalar.copy(
                                        tiles_T[h][:, o * 128:(o + 1) * 128],
                                        ps[:])
                                else:
                                    nc.vector.tensor_copy(
                                        tiles_T[h][:, o * 128:(o + 1) * 128],
                                        ps[:])

                # ---- x side ----
                xrB = fwd_side(lambda o: xk[o * 128:(o + 1) * 128, :], BT,
                               "x")
                scale_trigger(xrB, "x", sx_i, sx_o)
                sgx, rbx = scale_finish("x", sx_o)
                xT = [qTp.tile([128, B], BF, tag="qT", name=f"xT{h}")
                      for h in range(2)]
                quant_transpose(
                    xrB, rbx,
                    lambda c: nk[c * 512:(c + 1) * 512, :]
                    .rearrange("(o p) c -> p o c", p=128), "x", xT)
                for h in range(2):
                    nc.sync.dma_start(xqc[h * 128:(h + 1) * 128, :],
                                      xT[h][:])

                # ---- w side ----
                wrB = fwd_side(lambda o: wk[o * 128:(o + 1) * 128, :],
                               F // 128, "w")
                # AR-w first on the collective queue, AG right behind it
                scale_trigger(wrB, "w", sw_i, sw_o)
                if KSTOP >= 5:
                    nc.gpsimd.collective_compute(
                        "AllGather", A.bypass, replica_groups=rg,
                        ins=[xqc.ap().opt()], outs=[xqg.ap().opt()])
                sgw, rbw = scale_finish("w", sw_o)
                wT = [qTp.tile([128, B], BF, tag="qT", name=f"wT{h}")
                      for h in range(2)]
                quant_transpose(
                    wrB, rbw,
                    lambda c: mk[c * 512:(c + 1) * 512, :]
                    .rearrange("(o p) r -> p o r", p=128), "w", wT)
                for a in range(NCORES):
                    for h in range(2):
                        nc.sync.dma_start(
                            wac[a * CS + h * 128:a * CS + (h + 1) * 128, :],
                            wT[h][:, a * FS:(a + 1) * FS])

                # alpha = sx*sw/(QMAX^2 * 2^24)  (before the A2A trigger)
                al = qsc.tile([1, 1], DT, tag="al", name="al")
                nc.vector.tensor_tensor(al[0:1, 0:1], sgx[0:1, 0:1],
                                        sgw[0:1, 0:1], op=A.mult)
                nc.vector.tensor_scalar_mul(
                    al[0:1, 0:1], al[0:1, 0:1],
                    float(1.0 / (QMAX * QMAX * (1 << 24))))
                alb = qsc.tile([128, 1], DT, tag="alb", name="alb")
                nc.gpsimd.partition_broadcast(alb[:, 0:1], al[0:1, 0:1])
                if KSTOP >= 5:
                    nc.gpsimd.collective_compute(
                        "AllToAll", A.bypass, replica_groups=rg,
                        ins=[wac.ap().opt()], outs=[wblk.ap().opt()])

            if KSTOP < 6:
                raise _StopBuild()

            # ================= GEMM + inverse transforms =================
            with tc.tile_pool(name="yrp", bufs=1) as yrp, \
                 tc.tile_pool(name="gps", bufs=1, space="PSUM") as gps:
                yrb = yrp.tile([128, BT * FS], BF, tag="yrb", name="yrb")
                with tc.tile_pool(name="gem", bufs=KT) as gem:
                    xs, ws = [], []
                    for kt in range(KT):
                        tx = gem.tile([128, B], BF, tag="xs", name="xst")
                        nc.sync.dma_start(tx[:],
                                          xqg[kt * 128:(kt + 1) * 128, :])
                        xs.append(tx)
                        tw_ = gem.tile([128, FS], BF, tag="ws", name="wst")
                        nc.sync.dma_start(tw_[:],
                                          wblk[kt * 128:(kt + 1) * 128, :])
                        ws.append(tw_)
                    # kt-outer within groups of 8 batch tiles: GEMM starts
                    # as soon as the first k chunks land
                    for g in range(BT // 8):
                        pss = [gps.tile([128, FS], DT, tag="gp",
                                        name=f"gpt{g}_{i}", bufs=8)
                               for i in range(8)]
                        for kt in range(KT):
                            for i in range(8):
                                bo = g * 8 + i
                                nc.tensor.matmul(
                                    pss[i][:],
                                    xs[kt][:, bo * 128:(bo + 1) * 128],
                                    ws[kt][:], start=(kt == 0),
                                    stop=(kt == KT - 1))
                        for i in range(8):
                            bo = g * 8 + i
                            nc.vector.tensor_scalar(
                                yrb[:, bo * FS:(bo + 1) * FS], pss[i][:],
                                alb[:, 0:1], None, op0=A.mult)

                if KSTOP < 7:
                    raise _StopBuild()

                with tc.tile_pool(name="inv2", bufs=1) as invp:
                    # batch inverse: H128 per tile (in-place) + H32 butterfly
                    uB = invp.tile([128, BT * FS], BF, tag="gb", name="uB",
                                   bufs=2)
                    for bo in range(BT):
                        ps = gps.tile([128, FS], DT, tag="gp", name="gpt2",
                                      bufs=8)
                        nc.tensor.matmul(ps[:], h128b[:],
                                         yrb[:, bo * FS:(bo + 1) * FS],
                                         start=True, stop=True)
                        nc.vector.tensor_copy(yrb[:, bo * FS:(bo + 1) * FS],
                                              ps[:])
                    butterfly(nc, lambda s: (yrb, uB) if s % 2 == 0
                              else (uB, yrb), 32, FS, A)
                    # result in uB

                    # transpose [b-part, f] -> [f-part, b]
                    uT = [invp.tile([128, B], BF, tag="uT", name=f"uT{i}",
                                    bufs=4) for i in range(FT)]
                    for bo in range(BT):
                        for ft in range(FT):
                            ps = gps.tile([128, 128], BF, tag="gp",
                                          name="tpt", bufs=8)
                            nc.tensor.transpose(
                                ps[:],
                                uB[:, bo * FS + ft * 128:
                                   bo * FS + (ft + 1) * 128], idb[:])
                            nc.vector.tensor_copy(
                                uT[ft][:, bo * 128:(bo + 1) * 128], ps[:])

                    if KSTOP < 8:
                        raise _StopBuild()

                    # feature inverse: H128 per f-tile + H4 butterfly
                    zb = invp.tile([128, FT * B], BF, tag="gb", name="zb",
                                   bufs=2)
                    for ft in range(FT):
                        for nb in range(B // 512):
                            ps = gps.tile([128, 512], DT, tag="gp",
                                          name="zpt", bufs=8)
                            nc.tensor.matmul(
                                ps[:], h128b[:],
                                uT[ft][:, nb * 512:(nb + 1) * 512],
                                start=True, stop=True)
                            nc.scalar.copy(
                                zb[:, ft * B + nb * 512:
                                   ft * B + (nb + 1) * 512], ps[:])
                    z2 = invp.tile([128, FT * B], BF, tag="gb", name="z2",
                                   bufs=2)
                    # H4 stage 0: pairs (0,1),(2,3)
                    for h in range(2):
                        a0, a1 = h * 2 * B, h * 2 * B + B
                        nc.vector.tensor_tensor(z2[:, a0:a0 + B],
                                                zb[:, a0:a0 + B],
                                                zb[:, a1:a1 + B], op=A.add)
                        nc.vector.tensor_tensor(z2[:, a1:a1 + B],
                                                zb[:, a0:a0 + B],
                                                zb[:, a1:a1 + B],
                                                op=A.subtract)
                    # H4 stage 1 (pairs (0,2),(1,3)): fp32 chunks + DMA out
                    CB = 2048
                    for t in range(2):
                        for ft_o, sgn in ((t, A.add), (t + 2, A.subtract)):
                            for cb in range(B // CB):
                                vch = invp.tile([128, CB], DT, tag="vch",
                                                name="vch", bufs=4)
                                nc.vector.tensor_tensor(
                                    vch[:],
                                    z2[:, t * B + cb * CB:
                                       t * B + (cb + 1) * CB],
                                    z2[:, (t + 2) * B + cb * CB:
                                       (t + 2) * B + (cb + 1) * CB],
                                    op=sgn)
                                nc.sync.dma_start(
                                    out[ft_o * 128:(ft_o + 1) * 128,
                                        cb * CB:(cb + 1) * CB], vch[:])
      except _StopBuild:
        pass
    nc.compile()
    return nc


def kernel(**inputs):
    from concourse.bass_utils import run_bass_kernel_spmd

    if "nc" not in _cache:
        _cache["nc"] = _build()
    nc = _cache["nc"]

    x = np.asarray(inputs["inputs"], np.float32)
    w = np.asarray(inputs["kernel"], np.float32)
    bias = np.asarray(inputs["bias"], np.float32)
    nxp = 0.5 - np.asarray(inputs["noise_x"], np.float32)
    nwp = 0.5 - np.asarray(inputs["noise_w"], np.float32)

    in_maps = []
    for k in range(NCORES):
        cs = slice(k * CS, (k + 1) * CS)
        in_maps.append({
            "xk": np.ascontiguousarray(x[:, cs]),
            "nk": np.ascontiguousarray(nxp[:, cs]),
            "wk": np.ascontiguousarray(w[cs, :].T),
            "mk": np.ascontiguousarray(nwp[cs, :].T),
        })

    res = run_bass_kernel_spmd(nc, in_maps, list(range(NCORES)))
    V = np.stack([r["out"] for r in res.results])          # [a', g, b]
    H8 = _sylvester(8)
    yT = (H8 @ V.reshape(NCORES, -1)).reshape(F, B)        # [f, b], f=a*512+g
    y = np.ascontiguousarray(yT.T) + bias[None, :]
    return y.astype(np.float32)

